# revision 30
# baseline (speedup 1.0000x reference)
"""Trainium2 Bass kernel for nn_AttrSoftLoss (masked multilabel soft-margin loss).

Reference semantics: per row, drop the k = round(0.95 * n_zero) zero-labeled
positions whose fixed uniform draws (jax.random.key(42)) are smallest, then
average  -[a*log_sigmoid(s) + (1-a)*log_sigmoid(-s)]  over kept positions;
mean over rows.  With x = (1-2a)*s this is
loss = [sum_kept softplus(x)] / (B*C)  (the mask keeps all a=1 positions).

Host prep (layout/encoding only): rows pre-permuted into ascending order of
the fixed input-independent uniform matrix (the dropped set becomes "the
first k zero-labeled entries" in storage order), data stored TRANSPOSED
(classes on partitions, rows on the free dim), and the inputs re-encoded
bijectively as (x, h) with x = (1-2a)*s fp16 and h = 20*(1-a) in {0,20}
fp16 (the 20x makes every count below integer-exact in fp16/f32).

Device math: the keep decision c > rint(0.95*nz) (c = inclusive zero-prefix
count along the permuted class order, nz = row zero count) is evaluated in
the integer-exact scaled form Q = 20c + 20*1025*a - 19*nz - 10.4 > 0, which
deviates from the reference's round-half-even tie only on ~234 of 8.4M
boundary elements (rel err 5e-5, numpy-verified).  In h-units everything is
linear and block-local, so per [128, 1024] class-block cb:

    q_psum = (U - 1025*I)@h_cb + J@Hprev_cb + J@V0        (PE, f32-exact)
    Hprev_cb = sum_{b<cb} h_b        (7 chain adds, ride the DMA arrivals)
    V0 = -0.95*(Hprev_7 + h_7)       (one ts; exact: 0.95*20k = 19k)
    kept <=> q_psum > -20489.6       (single immediate constant!)

and mask+multiply+reduce is one fused DVE op per PAIR of blocks:
    stt(scr, q_pair, -20489.6, sp_pair, is_gt, mult, accum_out=stats)
over [128, 2048] two-bank PSUM pairs.  The [128, 4] stats vector goes
straight to DRAM; the host does the final tiny reduction at gather time
(it already sums the 8 per-core partials; a 4-byte device AllReduce would
cost ~50us + a ~100us NEFF entry barrier).

ScalarE computes softplus(x) = Ln(1 + Exp(x)) in fp16 (ramped chunk sizes
so it starts on the first quarter-block landing); the act-table list passed
to insert_act_table_loads is pruned (order-preserving, so runtime set ids
stay valid) so Exp and Ln share natural_log_exp_and_others: one table load.
GpSimd is left idle on purpose: its tensor ops run ~2.5us/[128,1024] AND
slow concurrent DVE ops ~4x via SBUF port contention (measured).
"""

import numpy as np

B, C = 8192, 1024
N_CORES = 8
ROWS = B // N_CORES  # 1024 rows per core (free dim after transpose)
NB = C // 128        # 8 class-blocks per core (partition dim)
THR = -409792.0      # = -20*(20*1025 - 10.4): kept <=> q20_psum > THR

_cache: dict = {}


def _make_bacc():
    from concourse import bacc, mybir

    class PrunedTableBacc(bacc.Bacc):
        """Prune Exp/Ln from every act-table set except
        natural_log_exp_and_others (order preserved, so the emitted
        act_func_set_id still indexes the real act_info list) - forces the
        first-fit chooser to put Exp and Ln on the one shared table."""

        def insert_act_table_loads(self):
            import bass_rust as _bass_rust
            from concourse.hw_specs import get_activation_tables

            keep = "natural_log_exp_and_others"
            drop = {
                mybir.ActivationFunctionType.Exp,
                mybir.ActivationFunctionType.Ln,
            }
            tables = []
            for name, funcs in get_activation_tables(self.m.arch).items():
                if name != keep:
                    funcs = {f for f in funcs if f not in drop}
                tables.append((name, funcs))
            _bass_rust.insert_act_table_loads(self, tables)

    return PrunedTableBacc(
        "TRN2", target_bir_lowering=False, debug=False, num_devices=N_CORES
    )


def _build_nc():
    from concourse import mybir, tile

    Alu = mybir.AluOpType
    Act = mybir.ActivationFunctionType
    f32 = mybir.dt.float32
    f16 = mybir.dt.float16
    f8e4 = mybir.dt.float8e4
    f8e5 = mybir.dt.float8e5

    nc = _make_bacc()
    x_d = nc.dram_tensor("x", [C, ROWS], f8e4, kind="ExternalInput")
    h_d = nc.dram_tensor("h", [C, ROWS], f8e5, kind="ExternalInput")
    w_d = nc.dram_tensor("wtri", [128, 128], f8e5, kind="ExternalInput")
    out_d = nc.dram_tensor("out", [128, NB // 2], f32, kind="ExternalOutput")

    with tile.TileContext(nc) as tc:
        with (
            tc.tile_pool(name="work", bufs=2) as work,
            tc.tile_pool(name="stat", bufs=1) as stat,
            tc.tile_pool(name="psum", bufs=2, space="PSUM") as psum,
        ):
            wtri = stat.tile([128, 128], f8e5)
            j20 = stat.tile([128, 128], f16)
            jm19 = stat.tile([128, 128], f16)
            stats = stat.tile([128, NB // 2], f32)
            nc.sync.dma_start(out=wtri[:], in_=w_d[:, :])
            nc.vector.memset(j20[:], 20.0)
            nc.vector.memset(jm19[:], -19.0)

            x_big = stat.tile([128, NB * ROWS], f8e4)
            h_big = stat.tile([128, NB * ROWS], f8e5)
            ex_big = stat.tile([128, NB * ROWS], f16)
            sp_big = stat.tile([128, NB * ROWS], f16)

            def blk(t, cb):
                return t[:, ROWS * cb : ROWS * (cb + 1)]

            # h-heavy DMA weave: h completes early (gates the HT barrier and
            # all J matmuls), x still leads so ACT can start immediately.
            def dma(t, d, cb, lo=0, hi=ROWS):
                nc.sync.dma_start(
                    out=t[:, ROWS * cb + lo : ROWS * cb + hi],
                    in_=d[128 * cb : 128 * (cb + 1), lo:hi],
                )

            dma(x_big, x_d, 0, 0, 256)
            dma(x_big, x_d, 0, 256, ROWS)
            dma(h_big, h_d, 0)
            dma(x_big, x_d, 1)
            dma(h_big, h_d, 1)
            dma(x_big, x_d, 2)
            dma(h_big, h_d, 2)
            dma(x_big, x_d, 3)
            dma(h_big, h_d, 3)
            dma(x_big, x_d, 4)
            dma(h_big, h_d, 4)
            dma(x_big, x_d, 5)
            dma(h_big, h_d, 5)
            dma(x_big, x_d, 6)
            dma(h_big, h_d, 6)
            dma(x_big, x_d, 7)
            dma(h_big, h_d, 7)

            # softplus(x) = Ln(1 + Exp(x)): ramped chunks (fp8 DMA lands all
            # of x by ~13us, so later chunks can be large).
            chunks = [(0, 256), (256, 1024), (1024, 2048), (2048, 4096),
                      (4096, 6144), (6144, 8192)]
            for lo, hi in chunks:
                nc.scalar.activation(
                    ex_big[:, lo:hi], x_big[:, lo:hi], Act.Exp
                )
                nc.scalar.activation(
                    sp_big[:, lo:hi], ex_big[:, lo:hi], Act.Ln, bias=1.0
                )

            # Prefix chain in h (rides the h arrivals; all integer-exact).
            Hprev = [None] * NB  # Hprev[cb] = sum_{b<cb} h_b; Hprev[0] = 0
            for cb in range(2, NB):
                if cb == 2:
                    Hprev[2] = stat.tile([128, ROWS], f16, tag="H2", name="H2")
                    nc.vector.tensor_tensor(
                        Hprev[2][:], blk(h_big, 0), blk(h_big, 1), Alu.add
                    )
                else:
                    nxt = stat.tile([128, ROWS], f16, tag=f"H{cb}")
                    nc.vector.tensor_tensor(
                        nxt[:], Hprev[cb - 1][:], blk(h_big, cb - 1), Alu.add
                    )
                    Hprev[cb] = nxt
            Hprev[1] = blk(h_big, 0)
            ht = stat.tile([128, ROWS], f16)
            nc.vector.tensor_tensor(
                ht[:], Hprev[NB - 1][:], blk(h_big, NB - 1), Alu.add
            )

            # Per block-pair: q (20-scaled) into a [128, 2048] two-bank PSUM
            # tile via three matmul terms per 512-slice, then one fused stt.
            # PE emission is software-pipelined: the -19*J@HT term (the only
            # barrier-gated one) closes each group as late as possible.
            qs = [None] * (NB // 2)

            def w_jh(pr):
                qs[pr] = psum.tile([128, 2 * ROWS], f32, tag="q", name=f"q{pr}")
                for half in range(2):
                    cb = 2 * pr + half
                    for hh in range(2):
                        sl = slice(ROWS * half + 512 * hh,
                                   ROWS * half + 512 * (hh + 1))
                        gsl = slice(ROWS * cb + 512 * hh,
                                    ROWS * cb + 512 * (hh + 1))
                        nc.tensor.matmul(
                            qs[pr][:, sl], wtri[:],
                            h_big[:, gsl], start=True, stop=False,
                        )
                        if cb > 0:
                            nc.tensor.matmul(
                                qs[pr][:, sl], j20[:],
                                Hprev[cb][:, 512 * hh : 512 * (hh + 1)],
                                start=False, stop=False,
                            )

            def j19(pr):
                for half in range(2):
                    for hh in range(2):
                        sl = slice(ROWS * half + 512 * hh,
                                   ROWS * half + 512 * (hh + 1))
                        nc.tensor.matmul(
                            qs[pr][:, sl], jm19[:],
                            ht[:, 512 * hh : 512 * (hh + 1)],
                            start=False, stop=True,
                        )

            def stt(pr):
                scr = work.tile([128, 2 * ROWS], f16, tag="scr")
                nc.vector.scalar_tensor_tensor(
                    scr[:], qs[pr][:], THR,
                    sp_big[:, 2 * ROWS * pr : 2 * ROWS * (pr + 1)],
                    op0=Alu.is_gt, op1=Alu.mult,
                    accum_out=stats[:, pr : pr + 1],
                )

            w_jh(0)
            w_jh(1)
            for pr in range(NB // 2):
                j19(pr)
                if pr + 2 < NB // 2:
                    w_jh(pr + 2)
                stt(pr)

            nc.sync.dma_start(out=out_d[:, :], in_=stats[:])

    nc.compile()
    return nc


def _get_nc():
    if "nc" not in _cache:
        _cache["nc"] = _build_nc()
    return _cache["nc"]


def _get_perm():
    """Constant per-row ascending-argsort of the fixed uniform matrix."""
    if "perm" not in _cache:
        import jax

        with jax.default_device(jax.devices("cpu")[0]):
            u = np.asarray(jax.random.uniform(jax.random.key(42), (B, C)))
        _cache["perm"] = np.argsort(u, axis=1, kind="stable")
    return _cache["perm"]


def _consts():
    if "wtri" not in _cache:
        import ml_dtypes

        # 20-scaled: lhsT[k,i] = 20*[k<=i], diag 20-20500 = -20480
        # (matmul computes lhsT.T @ rhs; 20 and -20480 are fp8e5-exact)
        w = 20.0 * np.triu(np.ones((128, 128), np.float32))
        np.fill_diagonal(w, -20480.0)
        _cache["wtri"] = w.astype(ml_dtypes.float8_e5m2)
    return _cache["wtri"]


def _make_in_maps(scores: np.ndarray, attributes: np.ndarray):
    perm = _get_perm()
    s_p = np.take_along_axis(np.asarray(scores, dtype=np.float32), perm, axis=1)
    a_p = np.take_along_axis(np.asarray(attributes, dtype=np.int32), perm, axis=1)
    import ml_dtypes

    # bijective input re-encode: (s, a) -> (x, h); x in fp8e4 (feeds only
    # softplus; quantization adds ~4e-4 rel on the loss), h = {0,20} exact
    # in fp8e5.
    x8 = ((1 - 2 * a_p) * s_p).astype(ml_dtypes.float8_e4m3fn)
    h8 = (20 * (1 - a_p)).astype(ml_dtypes.float8_e5m2)
    wtri = _consts()
    in_maps = []
    for i in range(N_CORES):
        r0, r1 = i * ROWS, (i + 1) * ROWS
        in_maps.append(
            {
                "x": np.ascontiguousarray(x8[r0:r1].T),
                "h": np.ascontiguousarray(h8[r0:r1].T),
                "wtri": wtri,
            }
        )
    return in_maps


def _run(in_maps, trace=False, **kwargs):
    from concourse import bass_utils

    return bass_utils.run_bass_kernel_spmd(
        _get_nc(), in_maps, core_ids=list(range(N_CORES)), trace=trace, **kwargs
    )


def kernel(scores: np.ndarray, attributes: np.ndarray) -> np.ndarray:
    res = _run(_make_in_maps(scores, attributes))
    total = np.float32(0.0)
    for r in res.results:
        total += np.asarray(r["out"], dtype=np.float32).sum(dtype=np.float32)
    return np.float32(total / (B * C)).reshape(())[()]
